# revision 13
# baseline (speedup 1.0000x reference)
"""Trainium2 Bass kernel for nn_Decoder (dense MLP).

Computes out = relu(V @ W1 + b1) @ W2 + b2 for V [262144, 1024],
W1 [1024, 128], W2 [128, 4].

Strategy
--------
Data-parallel over 8 NeuronCores: V is sharded along rows (32768 rows per
core); the small weights are replicated. Each core's V shard is transposed
and blocked on the host to [128, ngroups, 8, 2048] so the contraction dim
lands on SBUF partitions with one fully-contiguous 16KB-per-partition DMA
descriptor per group — no on-chip transposes.

Per core the kernel computes h.T = W1.T @ V.T with a k-outer loop: for
each 2048-row group, each of the 8 stationary W1 k-chunks is loaded once
and streamed against all four 512-row column chunks (PSUM-accumulated
across k). ReLU(+b1) runs on the scalar engine (PSUM -> fp16 SBUF), then
out.T = W2.T @ h.T on the tensor engine (fp16), +b2 on the vector engine
into a [4, 2048] group buffer stored contiguously by the Pool engine.
The host transposes the gathered [4, 32768] outputs back.

Precision modes (KERNEL_MODE env var):
  f8  — V cast to fp8 e3m4 (1 byte/elem DMA), weights fp16.  Rel err
        ~1.4e-2 (tolerance is 2e-2): V's 4-bit mantissa dominates; the
        exact sim of this quantization on the real inputs measures 0.0140.
  f16 — V cast to fp16 (2 bytes/elem DMA), weights fp16. Rel err ~3e-4.
"""

import os
import sys

import numpy as np

for _p in ("/opt/trn_rl_repo", "/root/.axon_site/_ro/trn_rl_repo"):
    if os.path.isdir(_p) and _p not in sys.path:
        sys.path.insert(0, _p)

import concourse.bass as bass
import concourse.mybir as mybir
import concourse.tile as tile
from concourse import bacc
from concourse.bass_utils import run_bass_kernel_spmd

NCORES = 8
NN = 262144
IN_DIM = 1024
HIDDEN = 128
OUT_DIM = 4
R = NN // NCORES  # rows per core

P = 128           # SBUF partitions
KC = IN_DIM // P  # 8 k-chunks
CHUNK = 512       # rows per PSUM accumulation tile (one PSUM bank)
GROUP = 2048      # rows per DMA group / k-outer supergroup
NG = R // GROUP   # 16 groups
NU = GROUP // CHUNK  # 4 chunks per group
DATA_BUFS = 4     # prefetch depth for V-group tiles

MODE = os.environ.get("KERNEL_MODE", "f8")
SKIP_LDW = os.environ.get("KERNEL_SKIP_LDW", "1") == "1"

_last_results = None  # exposed for test harness (exec_time_ns etc.)


def _v_dtype(mode):
    return mybir.dt.float8e3 if mode == "f8" else mybir.dt.float16


def _dedupe_ldweights(nc):
    """Remove InstLdweights that reload the stationary weights already in
    the PE array (same AP as the previous load, only matmuls in between).

    The tile framework pairs every matmul with its own weight load; within
    a k-chunk block the 2nd..Nth loads are redundant (~95ns of PE time
    each). Only loads with no semaphore waits/updates of their own, edges
    that duplicate the kept load's, and no inbound references are removed
    — anything else is kept conservatively.
    """
    removed = 0
    pe = mybir.EngineType.PE
    for f in nc.m.functions:
        for blk in f.blocks:
            insts = blk.instructions
            if not any(type(i).__name__ == "InstMatmult" for i in insts):
                continue
            # names referenced as dependency targets anywhere in the block
            referenced = set()
            for ins in insts:
                for name, _info in ins.dependency_edges():
                    referenced.add(name)

            prev_key = None
            prev_edges = None
            to_remove = []
            for idx, ins in enumerate(insts):
                if ins.engine != pe:
                    continue
                t = type(ins).__name__
                if t == "InstLdweights":
                    a = ins.ins[0]
                    key = (
                        a.memref, a.offset, str(a.ap), str(a.dtype),
                        str(ins.tile_size), str(ins.tile_position),
                        str(ins.perf_mode), str(ins.is_transpose),
                    )
                    si = ins.sync_info
                    clean = (
                        (si is None or (not si.on_wait and not si.on_update))
                        and ins.name not in referenced
                    )
                    edges = frozenset(n for n, _ in ins.dependency_edges())
                    if (
                        key == prev_key
                        and clean
                        and prev_edges is not None
                        and edges <= prev_edges
                    ):
                        to_remove.append(idx)
                    else:
                        prev_key = key
                        prev_edges = edges
                elif t in ("InstMatmult", "InstEventSemaphore"):
                    pass
                else:
                    prev_key = None
                    prev_edges = None
            for idx in reversed(to_remove):
                del insts[idx]
            removed += len(to_remove)
    return removed


def build_nc(mode=MODE, rows=R):
    """Build the SPMD Bass program for one core."""
    f32 = mybir.dt.float32
    f16 = mybir.dt.float16
    vdt = _v_dtype(mode)

    nc = bacc.Bacc("TRN2")

    vt_d = nc.declare_dram_parameter("VT", [P, NG * KC * GROUP], vdt, isOutput=False)
    w1_d = nc.declare_dram_parameter("W1", [IN_DIM, HIDDEN], f16, isOutput=False)
    b1_d = nc.declare_dram_parameter("B1", [HIDDEN, 1], f32, isOutput=False)
    w2_d = nc.declare_dram_parameter("W2", [HIDDEN, OUT_DIM], f16, isOutput=False)
    b2_d = nc.declare_dram_parameter("B2", [OUT_DIM, 1], f32, isOutput=False)
    out_d = nc.declare_dram_parameter("OUT", [OUT_DIM, rows], f32, isOutput=True)

    with tile.TileContext(nc) as tc:
        with (
            tc.tile_pool(name="const", bufs=1) as cpool,
            tc.tile_pool(name="data", bufs=DATA_BUFS) as dpool,
            tc.tile_pool(name="hbuf", bufs=2 * NU) as hpool,
            tc.tile_pool(name="obuf", bufs=2) as obpool,
            tc.tile_pool(name="psum1", bufs=6, space="PSUM") as ppool,
            tc.tile_pool(name="psum2", bufs=2, space="PSUM") as opool,
        ):
            # --- constants (loaded once) ---
            w1_sb = cpool.tile([P, KC, HIDDEN], f16)
            nc.sync.dma_start(w1_sb[:], w1_d[:].rearrange("(c p) h -> p c h", p=P))
            b1_sb = cpool.tile([HIDDEN, 1], f32)
            nc.sync.dma_start(b1_sb[:], b1_d[:])
            w2_sb = cpool.tile([HIDDEN, OUT_DIM], f16)
            nc.sync.dma_start(w2_sb[:], w2_d[:])
            b2_sb = cpool.tile([OUT_DIM, 1], f32)
            nc.sync.dma_start(b2_sb[:], b2_d[:])

            vt_view = vt_d[:].rearrange("p (g c n) -> g p c n", g=NG, c=KC, n=GROUP)
            out_view = out_d[:].rearrange("o (g n) -> g o n", n=GROUP)

            # mm2 work for the previous group, delayed so the PE never
            # waits on the scalar-engine ReLU evacuation:
            #   pending = (hh tiles, o_sb buffer, group index)
            pending = None

            def emit_mm2_step(pend, u):
                hh_tiles, o_sb, _g = pend
                po = opool.tile([OUT_DIM, CHUNK], f32, tag="po")
                nc.tensor.matmul(po[:], w2_sb[:], hh_tiles[u][:], start=True, stop=True)
                nc.vector.tensor_scalar_add(
                    o_sb[:, u * CHUNK : (u + 1) * CHUNK], po[:], b2_sb[:]
                )

            def flush_mm2(pend):
                hh_tiles, o_sb, g = pend
                nc.gpsimd.dma_start(out_view[g], o_sb[:])

            for g in range(NG):
                vt = dpool.tile([P, KC, GROUP], vdt, tag="vt")
                if g == 0:
                    # split the first group per k-chunk so the PE starts
                    # after ~1/8 of the group load
                    for c in range(KC):
                        nc.sync.dma_start(vt[:, c, :], vt_view[g][:, c, :])
                elif g % 2 == 1:
                    # alternate queues so group loads overlap at startup
                    nc.gpsimd.dma_start(vt[:], vt_view[g])
                else:
                    nc.sync.dma_start(vt[:], vt_view[g])

                ps_tiles = [
                    ppool.tile([HIDDEN, CHUNK], f32, tag="ps", name=f"ps{u}")
                    for u in range(NU)
                ]
                for c in range(KC):
                    w_ap = w1_sb[:, c, :]
                    for u in range(NU):
                        bi = nc.tensor.matmul(
                            ps_tiles[u][:],
                            w_ap,
                            vt[:, c, u * CHUNK : (u + 1) * CHUNK],
                            start=(c == 0),
                            stop=(c == KC - 1),
                        )
                        del bi  # weight-load dedupe happens post-build
                    # interleave the previous group's tiny layer-2 matmuls
                    # between k-chunks (one per chunk, c=1..NU)
                    if pending is not None and 1 <= c <= NU:
                        emit_mm2_step(pending, c - 1)
                if pending is not None:
                    flush_mm2(pending)

                hh_tiles = []
                for u in range(NU):
                    hh = hpool.tile([HIDDEN, CHUNK], f16, tag="hh")
                    nc.scalar.activation(
                        hh[:], ps_tiles[u][:],
                        mybir.ActivationFunctionType.Relu,
                        bias=b1_sb[:],
                    )
                    hh_tiles.append(hh)
                o_sb = obpool.tile([OUT_DIM, GROUP], f32, tag="o")
                pending = (hh_tiles, o_sb, g)

            for u in range(NU):
                emit_mm2_step(pending, u)
            flush_mm2(pending)

    if SKIP_LDW:
        _dedupe_ldweights(nc)
    return nc


def kernel(V, W1, b1, W2, b2):
    global _last_results
    mode = MODE
    if mode == "f8":
        import ml_dtypes

        np_vdt = ml_dtypes.float8_e3m4
    else:
        np_vdt = np.float16

    V = np.asarray(V, dtype=np.float32)
    W1 = np.asarray(W1, dtype=np.float32)
    b1 = np.asarray(b1, dtype=np.float32)
    W2 = np.asarray(W2, dtype=np.float32)
    b2 = np.asarray(b2, dtype=np.float32)

    common = {
        "W1": W1.astype(np.float16),
        "B1": np.ascontiguousarray(b1.reshape(HIDDEN, 1)),
        "W2": W2.astype(np.float16),
        "B2": np.ascontiguousarray(b2.reshape(OUT_DIM, 1)),
    }

    from concurrent.futures import ThreadPoolExecutor

    def prep_shard(c):
        shard = V[c * R : (c + 1) * R]  # [R, IN_DIM]
        # [IN_DIM, R] -> (c, p, g, n) -> [P, NG, KC, GROUP], one contiguous
        # (KC*GROUP)-run per (partition, group)
        vt = shard.T.reshape(KC, P, NG, GROUP).transpose(1, 2, 0, 3)
        return vt.astype(np_vdt).reshape(P, NG * KC * GROUP)

    with ThreadPoolExecutor(NCORES) as ex:
        vts = list(ex.map(prep_shard, range(NCORES)))
    in_maps = []
    for c in range(NCORES):
        m = {"VT": vts[c]}
        m.update(common)
        in_maps.append(m)

    nc = build_nc(mode, R)
    nc.finalize()
    res = run_bass_kernel_spmd(nc, in_maps, list(range(NCORES)))
    _last_results = res

    out = np.concatenate(
        [np.asarray(r["OUT"]).T for r in res.results], axis=0
    ).astype(np.float32)
    return out


# revision 18
# speedup vs baseline: 1.0326x; 1.0326x over previous
"""Trainium2 Bass kernel for nn_Decoder (dense MLP).

Computes out = relu(V @ W1 + b1) @ W2 + b2 for V [262144, 1024],
W1 [1024, 128], W2 [128, 4].

Strategy
--------
Data-parallel over 8 NeuronCores: V is sharded along rows (32768 rows per
core); the small weights are replicated. Each core's V shard is transposed
and blocked on the host to [128, ngroups, 8, 2048] so the contraction dim
lands on SBUF partitions with one fully-contiguous 16KB-per-partition DMA
descriptor per group — no on-chip transposes.

Per core the kernel computes h.T = W1.T @ V.T with a k-outer loop: for
each 2048-row group, each of the 8 stationary W1 k-chunks is loaded once
and streamed against all four 512-row column chunks (PSUM-accumulated
across k). ReLU(+b1) runs on the scalar engine (PSUM -> fp16 SBUF), then
out.T = W2.T @ h.T on the tensor engine (fp16), +b2 on the vector engine
into a [4, 2048] group buffer stored contiguously by the Pool engine.
The host transposes the gathered [4, 32768] outputs back.

Precision modes (KERNEL_MODE env var):
  f8  — V cast to fp8 e3m4 (1 byte/elem DMA), weights fp16.  Rel err
        ~1.4e-2 (tolerance is 2e-2): V's 4-bit mantissa dominates; the
        exact sim of this quantization on the real inputs measures 0.0140.
  f16 — V cast to fp16 (2 bytes/elem DMA), weights fp16. Rel err ~3e-4.
"""

import os
import sys

import numpy as np

for _p in ("/opt/trn_rl_repo", "/root/.axon_site/_ro/trn_rl_repo"):
    if os.path.isdir(_p) and _p not in sys.path:
        sys.path.insert(0, _p)

import concourse.bass as bass
import concourse.mybir as mybir
import concourse.tile as tile
from concourse import bacc
from concourse.bass_utils import run_bass_kernel_spmd

NCORES = 8
NN = 262144
IN_DIM = 1024
HIDDEN = 128
OUT_DIM = 4
R = NN // NCORES  # rows per core

P = 128           # SBUF partitions
KC = IN_DIM // P  # 8 k-chunks
CHUNK = 512       # rows per PSUM accumulation tile (one PSUM bank)
GROUP = 2048      # rows per DMA group / k-outer supergroup
NG = R // GROUP   # 16 groups
NU = GROUP // CHUNK  # 4 chunks per group
DATA_BUFS = 4     # prefetch depth for V-group tiles

MODE = os.environ.get("KERNEL_MODE", "f8")
SKIP_LDW = os.environ.get("KERNEL_SKIP_LDW", "1") == "1"

_last_results = None  # exposed for test harness (exec_time_ns etc.)


def _v_dtype(mode):
    return mybir.dt.float8e3 if mode == "f8" else mybir.dt.float16


def _dedupe_ldweights(nc):
    """Remove InstLdweights that reload the stationary weights already in
    the PE array (same AP as the previous load, only matmuls in between).

    The tile framework pairs every matmul with its own weight load; within
    a k-chunk block the 2nd..Nth loads are redundant (~95ns of PE time
    each). Only loads with no semaphore waits/updates of their own, edges
    that duplicate the kept load's, and no inbound references are removed
    — anything else is kept conservatively.
    """
    removed = 0
    pe = mybir.EngineType.PE
    for f in nc.m.functions:
        for blk in f.blocks:
            insts = blk.instructions
            if not any(type(i).__name__ == "InstMatmult" for i in insts):
                continue
            # names referenced as dependency targets anywhere in the block
            referenced = set()
            for ins in insts:
                for name, _info in ins.dependency_edges():
                    referenced.add(name)

            prev_key = None
            prev_edges = None
            to_remove = []
            for idx, ins in enumerate(insts):
                if ins.engine != pe:
                    continue
                t = type(ins).__name__
                if t == "InstLdweights":
                    a = ins.ins[0]
                    key = (
                        a.memref, a.offset, str(a.ap), str(a.dtype),
                        str(ins.tile_size), str(ins.tile_position),
                        str(ins.perf_mode), str(ins.is_transpose),
                    )
                    si = ins.sync_info
                    clean = (
                        (si is None or (not si.on_wait and not si.on_update))
                        and ins.name not in referenced
                    )
                    edges = frozenset(n for n, _ in ins.dependency_edges())
                    if (
                        key == prev_key
                        and clean
                        and prev_edges is not None
                        and edges <= prev_edges
                    ):
                        to_remove.append(idx)
                    else:
                        prev_key = key
                        prev_edges = edges
                elif t in (
                    "InstMatmult", "InstEventSemaphore", "InstDrain",
                    "InstNotify", "InstNop",
                ):
                    # none of these clobber the PE stationary array
                    pass
                else:
                    prev_key = None
                    prev_edges = None
            for idx in reversed(to_remove):
                del insts[idx]
            removed += len(to_remove)
    return removed


def build_nc(mode=MODE, rows=R):
    """Build the SPMD Bass program for one core."""
    f32 = mybir.dt.float32
    f16 = mybir.dt.float16
    vdt = _v_dtype(mode)

    nc = bacc.Bacc("TRN2")

    vt_d = nc.declare_dram_parameter("VT", [P, NG * KC * GROUP], vdt, isOutput=False)
    # W1 pre-blocked on host to [P, KC*HIDDEN] (2KB contiguous per partition)
    w1_d = nc.declare_dram_parameter("W1", [P, KC * HIDDEN], f16, isOutput=False)
    b1_d = nc.declare_dram_parameter("B1", [HIDDEN, 1], f32, isOutput=False)
    w2_d = nc.declare_dram_parameter("W2", [HIDDEN, OUT_DIM], f16, isOutput=False)
    b2_d = nc.declare_dram_parameter("B2", [OUT_DIM, 1], f32, isOutput=False)
    out_d = nc.declare_dram_parameter("OUT", [OUT_DIM, rows], f32, isOutput=True)

    with tile.TileContext(nc) as tc:
        with (
            tc.tile_pool(name="const", bufs=1) as cpool,
            tc.tile_pool(name="data", bufs=DATA_BUFS) as dpool,
            tc.tile_pool(name="hbuf", bufs=2 * NU) as hpool,
            tc.tile_pool(name="obuf", bufs=2) as obpool,
            tc.tile_pool(name="psum1", bufs=6, space="PSUM") as ppool,
            tc.tile_pool(name="psum2", bufs=2, space="PSUM") as opool,
        ):
            vt_view = vt_d[:].rearrange("p (g c n) -> g p c n", g=NG, c=KC, n=GROUP)
            out_view = out_d[:].rearrange("o (g n) -> g o n", n=GROUP)

            # --- first V chunk before anything else so the PE can start;
            # constants that only gate later stages go on the scalar queue
            vt0 = dpool.tile([P, KC, GROUP], vdt, tag="vt", name="vt0")
            nc.sync.dma_start(vt0[:, 0, :], vt_view[0][:, 0, :])
            w1_sb = cpool.tile([P, KC, HIDDEN], f16)
            nc.sync.dma_start(w1_sb[:], w1_d[:].rearrange("p (c h) -> p c h", c=KC))
            b1_sb = cpool.tile([HIDDEN, 1], f32)
            nc.scalar.dma_start(b1_sb[:], b1_d[:])
            w2_sb = cpool.tile([HIDDEN, OUT_DIM], f16)
            nc.scalar.dma_start(w2_sb[:], w2_d[:])
            b2_sb = cpool.tile([OUT_DIM, 1], f32)
            nc.scalar.dma_start(b2_sb[:], b2_d[:])

            # mm2 work for the previous group, delayed so the PE never
            # waits on the scalar-engine ReLU evacuation:
            #   pending = (hh tiles, o_sb buffer, group index)
            pending = None

            def emit_mm2_step(pend, u):
                hh_tiles, o_sb, _g = pend
                po = opool.tile([OUT_DIM, CHUNK], f32, tag="po")
                nc.tensor.matmul(po[:], w2_sb[:], hh_tiles[u][:], start=True, stop=True)
                nc.vector.tensor_scalar_add(
                    o_sb[:, u * CHUNK : (u + 1) * CHUNK], po[:], b2_sb[:]
                )

            def flush_mm2(pend):
                hh_tiles, o_sb, g = pend
                nc.gpsimd.dma_start(out_view[g], o_sb[:])

            for g in range(NG):
                if g == 0:
                    # split the first group per k-chunk (c=0 already issued
                    # above) across both queues so the PE starts early
                    vt = vt0
                    for c in range(1, KC):
                        eng = nc.sync if c % 2 == 0 else nc.gpsimd
                        eng.dma_start(vt[:, c, :], vt_view[g][:, c, :])
                else:
                    vt = dpool.tile([P, KC, GROUP], vdt, tag="vt")
                    # alternate queues so group loads overlap at startup
                    eng = nc.gpsimd if g % 2 == 1 else nc.sync
                    eng.dma_start(vt[:], vt_view[g])

                ps_tiles = [
                    ppool.tile([HIDDEN, CHUNK], f32, tag="ps", name=f"ps{u}")
                    for u in range(NU)
                ]
                for c in range(KC):
                    w_ap = w1_sb[:, c, :]
                    for u in range(NU):
                        bi = nc.tensor.matmul(
                            ps_tiles[u][:],
                            w_ap,
                            vt[:, c, u * CHUNK : (u + 1) * CHUNK],
                            start=(c == 0),
                            stop=(c == KC - 1),
                        )
                        del bi  # weight-load dedupe happens post-build
                    # interleave the previous group's tiny layer-2 matmuls
                    # between k-chunks (one per chunk, c=1..NU)
                    if pending is not None and 1 <= c <= NU:
                        emit_mm2_step(pending, c - 1)
                if pending is not None:
                    flush_mm2(pending)

                hh_tiles = []
                for u in range(NU):
                    hh = hpool.tile([HIDDEN, CHUNK], f16, tag="hh")
                    nc.scalar.activation(
                        hh[:], ps_tiles[u][:],
                        mybir.ActivationFunctionType.Relu,
                        bias=b1_sb[:],
                    )
                    hh_tiles.append(hh)
                o_sb = obpool.tile([OUT_DIM, GROUP], f32, tag="o")
                pending = (hh_tiles, o_sb, g)

            for u in range(NU):
                emit_mm2_step(pending, u)
            flush_mm2(pending)

    if SKIP_LDW:
        _dedupe_ldweights(nc)
    return nc


def kernel(V, W1, b1, W2, b2):
    global _last_results
    mode = MODE
    if mode == "f8":
        import ml_dtypes

        np_vdt = ml_dtypes.float8_e3m4
    else:
        np_vdt = np.float16

    V = np.asarray(V, dtype=np.float32)
    W1 = np.asarray(W1, dtype=np.float32)
    b1 = np.asarray(b1, dtype=np.float32)
    W2 = np.asarray(W2, dtype=np.float32)
    b2 = np.asarray(b2, dtype=np.float32)

    # W1 [IN_DIM, H] -> [P, KC*H]: row k = c*P + p lands at [p, c*H:(c+1)*H]
    w1_blocked = np.ascontiguousarray(
        W1.astype(np.float16).reshape(KC, P, HIDDEN).transpose(1, 0, 2)
    ).reshape(P, KC * HIDDEN)
    common = {
        "W1": w1_blocked,
        "B1": np.ascontiguousarray(b1.reshape(HIDDEN, 1)),
        "W2": W2.astype(np.float16),
        "B2": np.ascontiguousarray(b2.reshape(OUT_DIM, 1)),
    }

    from concurrent.futures import ThreadPoolExecutor

    def prep_shard(c):
        shard = V[c * R : (c + 1) * R]  # [R, IN_DIM]
        # [IN_DIM, R] -> (c, p, g, n) -> [P, NG, KC, GROUP], one contiguous
        # (KC*GROUP)-run per (partition, group)
        vt = shard.T.reshape(KC, P, NG, GROUP).transpose(1, 2, 0, 3)
        return vt.astype(np_vdt).reshape(P, NG * KC * GROUP)

    with ThreadPoolExecutor(NCORES) as ex:
        vts = list(ex.map(prep_shard, range(NCORES)))
    in_maps = []
    for c in range(NCORES):
        m = {"VT": vts[c]}
        m.update(common)
        in_maps.append(m)

    nc = build_nc(mode, R)
    nc.finalize()
    res = run_bass_kernel_spmd(nc, in_maps, list(range(NCORES)))
    _last_results = res

    out = np.concatenate(
        [np.asarray(r["OUT"]).T for r in res.results], axis=0
    ).astype(np.float32)
    return out
